# revision 2
# baseline (speedup 1.0000x reference)
"""Causal self-attention Trainium2 kernel.

Problem: B=4, T=2048, D=1024, H=16 heads, Dh=64.
Sharding: 8 cores = 4 batches x 2 head-groups (8 heads/group).
  - data parallel over batch, tensor parallel over heads
    (qkv column-parallel, out_proj row-parallel; host sums the two
    partial outputs per batch and adds the bias row).

Per-core kernel (Tile framework, bf16 matmuls with fp32 PSUM accum):
  phase 1: QKV projection.
      Q^T, K^T stored [head_dim, T] (pair-packed: 2 heads -> 128 partitions)
      V stored [T, 8 heads x (64 dims + ones-col)]  (ones col -> softmax denom)
  phase 2: attention per (q-tile of 512, head-pair):
      S^T[k,q] = K^T-block.T @ Q^T  (row-packed 2 heads in the PE array)
      P^T = exp(S^T / 8)            (no max subtraction: logits bounded)
      causal: block skipping + 4 static multiplicative masks on P^T
      O^T[d,q] (+ denom row) = [V|1].T @ P^T  accumulated over k-blocks
      normalize: rden = 1/denom, PE-broadcast, multiply -> O^T_norm (bf16)
  phase 3: y_partial = O_norm @ W_out_rows  (accumulate over head pairs)
"""

import os
import sys

import numpy as np

sys.path.insert(0, "/opt/trn_rl_repo")

import ml_dtypes  # noqa: E402

import concourse.bass as bass  # noqa: E402
import concourse.mybir as mybir  # noqa: E402
import concourse.tile as tile  # noqa: E402
from concourse import bacc  # noqa: E402
from concourse.bass_utils import run_bass_kernel_spmd  # noqa: E402

BF16 = mybir.dt.bfloat16
F32 = mybir.dt.float32

B, T, D = 4, 2048, 1024
H, DH = 16, 64
G = 2                      # head groups (cores per batch)
HL = H // G                # heads per core (8)
CL = HL * DH               # local channel width (512)
NP = HL // 2               # head pairs per core (4)
QT = 512                   # query tile (free dim)
KB = 128                   # key block (psum partitions)
NQT = T // QT              # 4
NTT = T // 128             # 16 t-tiles of 128
NDB = D // 128             # 8 contraction blocks for projections
SCALE = 1.0 / 8.0          # 1/sqrt(DH)

_CACHE: dict = {}


def _build_program():
    nc = bacc.Bacc(
        "TRN2",
        target_bir_lowering=False,
        debug=False,
        num_devices=8,
    )

    xT_d = nc.dram_tensor("xT", [D, T], BF16, kind="ExternalInput")
    wq_d = nc.dram_tensor("wq", [D, CL], BF16, kind="ExternalInput")
    wk_d = nc.dram_tensor("wk", [D, CL], BF16, kind="ExternalInput")
    wv_d = nc.dram_tensor("wv", [D, CL], BF16, kind="ExternalInput")
    wo_d = nc.dram_tensor("wo", [CL, D], BF16, kind="ExternalInput")
    bqk_d = nc.dram_tensor("bqk", [2, NP, 128, 1], F32, kind="ExternalInput")
    mask_d = nc.dram_tensor("masks", [4, 128, QT], BF16, kind="ExternalInput")
    y_d = nc.dram_tensor("y", [T, D], F32, kind="ExternalOutput")

    Exp = mybir.ActivationFunctionType.Exp

    with tile.TileContext(nc) as tc:
        with (
            tc.tile_pool(name="const", bufs=1) as cpool,
            tc.tile_pool(name="xt", bufs=1) as xpool,
            tc.tile_pool(name="big", bufs=1) as bpool,
            tc.tile_pool(name="work", bufs=4) as wkpool,
            tc.tile_pool(name="out", bufs=2) as opool_sb,
            tc.tile_pool(name="ps_st", bufs=2, space="PSUM") as pst,
            tc.tile_pool(name="ps_o", bufs=2, space="PSUM") as pso,
            tc.tile_pool(name="ps_mm", bufs=2, space="PSUM") as pmm,
        ):
            # ---- persistent SBUF tiles -------------------------------------
            wq_sb = [cpool.tile([128, CL], BF16, tag=f"wq{i}", name=f"wq{i}")
                     for i in range(NDB)]
            wk_sb = [cpool.tile([128, CL], BF16, tag=f"wk{i}", name=f"wk{i}")
                     for i in range(NDB)]
            wv_sb = [cpool.tile([128, CL], BF16, tag=f"wv{i}", name=f"wv{i}")
                     for i in range(NDB)]
            wo_sb = [cpool.tile([128, D], BF16, tag=f"wo{i}", name=f"wo{i}")
                     for i in range(NP)]
            mask_sb = cpool.tile([128, 4, QT], BF16, tag="mask", name="mask_sb")
            bias_sb = [[cpool.tile([128, 1], F32, tag=f"b{j}{p}", name=f"b{j}{p}")
                        for p in range(NP)] for j in range(2)]
            ones_sb = cpool.tile([1, 64], F32, tag="ones", name="ones_sb")

            xT_sb = [xpool.tile([128, T], BF16, tag=f"x{i}", name=f"x{i}")
                     for i in range(NDB)]

            qt_sb = [bpool.tile([128, T], BF16, tag=f"q{p}", name=f"q{p}")
                     for p in range(NP)]
            kt_sb = [bpool.tile([128, T], BF16, tag=f"k{p}", name=f"k{p}")
                     for p in range(NP)]
            v_sb = [bpool.tile([128, HL, DH + 1], BF16, tag=f"v{i}", name=f"v{i}")
                    for i in range(NTT)]
            ot_sb = [bpool.tile([128, T], BF16, tag=f"o{p}", name=f"o{p}")
                     for p in range(NP)]

            # ---- input DMAs -------------------------------------------------
            for i in range(NDB):
                nc.sync.dma_start(wq_sb[i][:], wq_d[i * 128:(i + 1) * 128, :])
                nc.sync.dma_start(wk_sb[i][:], wk_d[i * 128:(i + 1) * 128, :])
                nc.sync.dma_start(wv_sb[i][:], wv_d[i * 128:(i + 1) * 128, :])
            for p in range(NP):
                nc.sync.dma_start(wo_sb[p][:], wo_d[p * 128:(p + 1) * 128, :])
            for j in range(4):
                nc.sync.dma_start(mask_sb[:, j, :], mask_d[j, :, :])
            for j in range(2):
                for p in range(NP):
                    nc.sync.dma_start(bias_sb[j][p][:], bqk_d[j, p, :, :])
            nc.vector.memset(ones_sb[:], 1.0)
            for i in range(NDB):
                nc.sync.dma_start(xT_sb[i][:], xT_d[i * 128:(i + 1) * 128, :])

            # ---- phase 1 helpers -------------------------------------------
            def emit_qkt(p):
                """Q^T and K^T for head pair p: [128 (2 heads x 64 dims), T]."""
                for which, wsb, dst in ((0, wq_sb, qt_sb[p]), (1, wk_sb, kt_sb[p])):
                    for t4 in range(NQT):
                        ps = pmm.tile([128, QT], F32, tag="mm",
                                      name=f"ps_qk{which}_{p}_{t4}")
                        for db in range(NDB):
                            nc.tensor.matmul(
                                ps[:],
                                wsb[db][:, p * 128:(p + 1) * 128],
                                xT_sb[db][:, t4 * QT:(t4 + 1) * QT],
                                start=(db == 0),
                                stop=(db == NDB - 1),
                            )
                        nc.vector.tensor_scalar_add(
                            dst[:, t4 * QT:(t4 + 1) * QT], ps[:],
                            bias_sb[which][p][:],
                        )

            def emit_v(tt):
                """V rows for t-tile tt -> v_sb[tt][:, h, 0:64]; col 64 = ones."""
                ps = pmm.tile([128, QT], F32, tag="mm", name=f"ps_v{tt}")
                for db in range(NDB):
                    nc.tensor.matmul(
                        ps[:],
                        xT_sb[db][:, tt * 128:(tt + 1) * 128],
                        wv_sb[db][:],
                        start=(db == 0),
                        stop=(db == NDB - 1),
                    )
                nc.vector.memset(v_sb[tt][:, :, DH:DH + 1], 1.0)
                nc.vector.tensor_copy(
                    v_sb[tt][:, :, 0:DH],
                    ps[:].rearrange("p (h d) -> p h d", h=HL),
                )

            # ---- phase 2: attention for (qt tile, head pair) ----------------
            def emit_attn(qt, p):
                nkb = 4 * (qt + 1)          # causal: key blocks 0..nkb-1
                ps_o = [pso.tile([DH + 1, QT], F32, tag="o", name=f"ps_o{qt}_{p}_{h}")
                        for h in range(2)]
                for kbp in range(nkb // 2):
                    for h in range(2):
                        rows = slice(64 * h, 64 * h + 64)
                        st = pst.tile([128, 2 * QT], F32, tag="st",
                                      name=f"st{qt}_{p}_{kbp}_{h}")
                        for j in range(2):
                            kb = 2 * kbp + j
                            nc.tensor.matmul(
                                st[:, j * QT:(j + 1) * QT],
                                kt_sb[p][rows, kb * KB:(kb + 1) * KB],
                                qt_sb[p][rows, qt * QT:(qt + 1) * QT],
                                start=True, stop=True,
                            )
                        pt = wkpool.tile([128, 2 * QT], BF16, tag="pt",
                                         name=f"pt{qt}_{p}_{kbp}_{h}")
                        nc.scalar.activation(pt[:], st[:], Exp, scale=SCALE)
                        for j in range(2):
                            kb = 2 * kbp + j
                            jd = kb - 4 * qt
                            if 0 <= jd <= 3:
                                nc.vector.tensor_mul(
                                    pt[:, j * QT:(j + 1) * QT],
                                    pt[:, j * QT:(j + 1) * QT],
                                    mask_sb[:, jd, :],
                                )
                        for j in range(2):
                            kb = 2 * kbp + j
                            nc.tensor.matmul(
                                ps_o[h][:],
                                v_sb[kb][:, 2 * p + h, :],
                                pt[:, j * QT:(j + 1) * QT],
                                start=(kb == 0),
                                stop=(kb == nkb - 1),
                            )
                # normalize: O^T[d,q] / denom[q]  -> ot_sb (bf16)
                for h in range(2):
                    rden = wkpool.tile([1, QT], F32, tag="rden",
                                       name=f"rden{qt}_{p}_{h}")
                    nc.vector.reciprocal(rden[:], ps_o[h][DH:DH + 1, :])
                    bc = pmm.tile([64, QT], F32, tag="mm", name=f"bc{qt}_{p}_{h}")
                    nc.tensor.matmul(bc[:], ones_sb[:], rden[:],
                                     start=True, stop=True)
                    bcs = opool_sb.tile([64, QT], F32, tag="bcs",
                                        name=f"bcs{qt}_{p}_{h}")
                    nc.vector.tensor_copy(bcs[:], bc[:])
                    nc.vector.tensor_mul(
                        ot_sb[p][64 * h:64 * h + 64, qt * QT:(qt + 1) * QT],
                        ps_o[h][0:DH, :],
                        bcs[:],
                    )

            # ---- phase 3: out-projection for the 4 t-tiles of qt ------------
            def emit_proj(qt):
                for ct in range(2):
                    for tt in range(4 * qt, 4 * qt + 4):
                        ps = pmm.tile([128, QT], F32, tag="mm",
                                      name=f"ps_y{qt}_{ct}_{tt}")
                        for p in range(NP):
                            nc.tensor.matmul(
                                ps[:],
                                ot_sb[p][:, tt * 128:(tt + 1) * 128],
                                wo_sb[p][:, ct * QT:(ct + 1) * QT],
                                start=(p == 0),
                                stop=(p == NP - 1),
                            )
                        ysb = opool_sb.tile([128, QT], F32, tag="ysb",
                                            name=f"ysb{qt}_{ct}_{tt}")
                        nc.vector.tensor_copy(ysb[:], ps[:])
                        nc.sync.dma_start(
                            y_d[tt * 128:(tt + 1) * 128, ct * QT:(ct + 1) * QT],
                            ysb[:],
                        )

            # ---- emission order (tuned so ScalarE's exp stream starts early
            # and PE keeps phase-1 work to fill exp-bound stretches) ----------
            emit_qkt(0)
            for tt in range(0, 4):
                emit_v(tt)
            emit_attn(0, 0)
            emit_qkt(1)
            emit_attn(0, 1)
            emit_qkt(2)
            emit_attn(0, 2)
            emit_qkt(3)
            emit_attn(0, 3)
            emit_proj(0)
            for qt in range(1, NQT):
                for tt in range(4 * qt, 4 * qt + 4):
                    emit_v(tt)
                for p in range(NP):
                    emit_attn(qt, p)
                emit_proj(qt)

    nc.compile()
    return nc


def _get_program():
    if "nc" not in _CACHE:
        _CACHE["nc"] = _build_program()
    return _CACHE["nc"]


def _causal_masks():
    """mask[j][k', q'] = 1.0 if (128*j + k') <= q' else 0  (bf16)."""
    kk = np.arange(128)[:, None]
    qq = np.arange(QT)[None, :]
    m = np.stack([(128 * j + kk <= qq) for j in range(4)])
    return m.astype(ml_dtypes.bfloat16)


def make_in_maps(x, w_qkv, b_qkv, w_out):
    bf16 = ml_dtypes.bfloat16
    masks = _causal_masks()
    in_maps = []
    for c in range(8):
        b, g = c // 2, c % 2
        cs = slice(CL * g, CL * (g + 1))
        bq = b_qkv[cs.start:cs.stop].reshape(NP, 128, 1).astype(np.float32)
        bk = b_qkv[D + cs.start:D + cs.stop].reshape(NP, 128, 1).astype(np.float32)
        in_maps.append({
            "xT": np.ascontiguousarray(x[b].T).astype(bf16),
            "wq": np.ascontiguousarray(w_qkv[:, cs]).astype(bf16),
            "wk": np.ascontiguousarray(w_qkv[:, D + cs.start:D + cs.stop]).astype(bf16),
            "wv": np.ascontiguousarray(
                w_qkv[:, 2 * D + cs.start:2 * D + cs.stop]).astype(bf16),
            "wo": np.ascontiguousarray(w_out[cs, :]).astype(bf16),
            "bqk": np.stack([bq, bk]).astype(np.float32),
            "masks": masks,
        })
    return in_maps


def kernel(x, w_qkv, b_qkv, w_out, b_out, _results_hook=None):
    x = np.asarray(x, dtype=np.float32)
    w_qkv = np.asarray(w_qkv, dtype=np.float32)
    b_qkv = np.asarray(b_qkv, dtype=np.float32)
    w_out = np.asarray(w_out, dtype=np.float32)
    b_out = np.asarray(b_out, dtype=np.float32)

    nc = _get_program()
    in_maps = make_in_maps(x, w_qkv, b_qkv, w_out)
    res = run_bass_kernel_spmd(nc, in_maps, list(range(8)))
    if _results_hook is not None:
        _results_hook(res)

    # host-side constant row: v-bias passes through softmax untouched
    # (attention rows sum to 1), then through the out projection.
    host_row = (
        b_qkv[2 * D:].astype(np.float64) @ w_out.astype(np.float64)
        + b_out.astype(np.float64)
    ).astype(np.float32)

    y = np.empty((B, T, D), dtype=np.float32)
    for b in range(B):
        y[b] = res.results[2 * b]["y"] + res.results[2 * b + 1]["y"] + host_row
    return y
